# revision 33
# baseline (speedup 1.0000x reference)
"""Trainium2 Bass kernel for nn_NodeAttentionPerMetaPath (GAT-style node attention).

Reference computation (N=8192, F_IN=256, d=64):
    h      = x @ trans                      # [N, d]
    e1     = h @ attn[:d];  e2 = h @ attn[d:]
    scores = leaky_relu(e1 + e2.T, 0.2)     # [N, N]
    masked = where(mask==0, -1e15, scores)
    out    = softmax(masked, axis=1) @ h    # [N, d]

Sharding: rows (r) across 8 cores, 1024 rows each. Every core computes the
full h locally from a streamed copy of x (no collectives at all).

Algebra (exp of leaky_relu as a max of two exponentials; the exp(a*e1) factor
cancels in the softmax ratio):
    P'[r,j] = m[r,j] * max(C[r]*D[j], 1),  C = exp((1-a)e1), D = exp((1-a)e2)
    out[r]  = (sum_j P'[r,j]*B2[j]*h[j]) / (sum_j P'[r,j]*B2[j]),
    B2 = exp(a*e2); B2*h and B2 live as columns of one lhsT so a single
    accumulated PE matmul yields numerator AND denominator.

Device data flow is [j, r] so NO [N,N] transpose is ever needed on-device:
    - host uploads maskT (mask transposed, fp16 0/1): j lands on partitions
    - v[j,r] = max(C[r]*D[j], 1): one DVE tensor_scalar (4x 16-bit mode)
    - P'T    = v * maskT in place: one DVE/GPSIMD tensor_tensor (packed fp16)
    - out.T  = accumulated PE matmul over 64 j-chunks, lhsT = [B2*h | B2]

Host-side packing (lossless or quantization-only input repacking):
    - x -> xT4: fp16, pre-transposed into [g, p, kk, fc, n] 4-chunk groups so
      PE weight loads read it directly (no device transposes)
    - mask -> maskT fp16 (0/1 exact; halves mask DMA vs int32)
    - rhs_f = [trans | trans@attn] fp16: each x chunk yields h AND e1/e2 in
      one accumulated matmul pair
    - per-core chunk rotation: core c sees its OWN 8 node-chunks first (c_rep
      is needed early); maskT rows and haug slots use the same rotated j
      order, harmless since sum_j is order-invariant.
"""

from contextlib import ExitStack

import numpy as np

import concourse.bass as bass
import concourse.bacc as bacc
import concourse.mybir as mybir
import concourse.tile as tile
from concourse.bass_utils import run_bass_kernel_spmd
from concourse.masks import make_identity

f32 = mybir.dt.float32
f16 = mybir.dt.float16

Exp = mybir.ActivationFunctionType.Exp
Ident = mybir.ActivationFunctionType.Identity

N_CORES = 8
N = 8192
F_IN = 256
D = 64  # F_OUT
ALPHA = 0.2

R = N // N_CORES  # own rows per core
JC = N // 128  # j-chunks
FC = F_IN // 128  # f-chunks
KG = 4  # j-chunks per x/he group
NG = JC // KG

# haug columns: 0:64 = B2*h, 64 = B2 (denominator), 65 = zero pad
# (fp16 matmul lhsT needs an even element count)
H_ONE = D
H_W = D + 2
HE_W = D + 2  # he columns: 0:64 h, 64 e1, 65 e2


def build_kernel(ctx: ExitStack, tc: tile.TileContext, xT4, maskT_rot, rhs_f, outT):
    nc = tc.nc

    singles = ctx.enter_context(tc.tile_pool(name="singles", bufs=1))
    xp = ctx.enter_context(tc.tile_pool(name="xp", bufs=3))
    maskp = ctx.enter_context(tc.tile_pool(name="maskp", bufs=3))
    vp = ctx.enter_context(tc.tile_pool(name="vp", bufs=4))
    gvp = ctx.enter_context(tc.tile_pool(name="gvp", bufs=1))
    ps_he = ctx.enter_context(tc.tile_pool(name="ps_he", bufs=2, space="PSUM"))
    ps_o = ctx.enter_context(tc.tile_pool(name="ps_o", bufs=1, space="PSUM"))
    outp = ctx.enter_context(tc.tile_pool(name="outp", bufs=1))

    rhs_sb = singles.tile([128, FC, HE_W], f16)
    nc.gpsimd.dma_start(
        out=rhs_sb, in_=rhs_f.rearrange("(c p) d -> p c d", p=128)
    )

    # ---- interleaved input streams: xT group g (256KB) then its 4 maskT
    # tiles (256KB each) so a chunk's h is always ready before its mask.
    # own-row x groups 0/1 feed the c_rep critical path: issue their
    # per-chunk DMAs FIRST (the sync queue issues ~1 DMA per 700ns)
    x_tiles = []
    m_tiles = []
    for g in range(2):
        xt = xp.tile([128, KG, FC, 128], f16, tag="x")
        for kk in range(KG):
            nc.sync.dma_start(out=xt[:, kk, :, :], in_=xT4[g, :, kk])
        x_tiles.append(xt)
    for g in range(2):
        mt = maskp.tile([128, KG, R], f16, tag="m")
        for kk in range(KG):
            k = g * KG + kk
            nc.sync.dma_start(
                out=mt[:, kk, :], in_=maskT_rot[k * 128:(k + 1) * 128, :]
            )
        m_tiles.append(mt)
    for g in range(2, NG):
        xt = xp.tile([128, KG, FC, 128], f16, tag="x")
        nc.sync.dma_start(out=xt, in_=xT4[g])
        x_tiles.append(xt)
        # octo mask tile shared by group pairs; DMA stays quad-granular
        if g % 2 == 0:
            mt2 = maskp.tile([128, 2 * KG, R], f16, tag="m2")
            m_tiles.append(mt2)
        else:
            mt2 = m_tiles[-1]
            m_tiles.append(mt2)
        half = g % 2
        nc.sync.dma_start(
            out=mt2[:, half * KG:(half + 1) * KG, :],
            in_=maskT_rot[g * KG * 128:(g + 1) * KG * 128, :].rearrange(
                "(kk p) r -> p kk r", p=128
            ),
        )

    # pin the natural_log_exp_and_others ACT table (id 6) at boot: it covers
    # every func used here (Exp/Identity/Copy/Ln) so no mid-run table swaps
    nc.scalar.add_instruction(
        mybir.InstLoadActFuncSet(
            name=nc.get_next_instruction_name(), ins=[], outs=[], act_func_set_id=6
        )
    )
    ident = singles.tile([128, 128], f16)
    make_identity(nc, ident)
    ones128 = singles.tile([128, 128], f16)
    nc.vector.memset(ones128, 1.0)
    ones_row_f = singles.tile([1, D], f32)
    nc.vector.memset(ones_row_f, 1.0)

    haug = singles.tile([128, JC, H_W], f16)
    nc.vector.memset(haug[:, :, H_ONE + 1], 0.0)
    # f32 per-partition scalars: D (for the tensor_scalar), B2 (ACT scale), C
    scl_d = singles.tile([128, JC], f32)
    scl_b2 = singles.tile([128, JC], f32)
    scl_c = singles.tile([128, 16], f32)
    c_rep = singles.tile([128, R], f16)

    po = ps_o.tile([D + 2, R], f32)

    v_tiles = {}
    GP_GROUPS = ()  # (gpsimd TT routing measured harmful; keep empty)
    ACC_ORDER = list(range(NG))

    def attention_dve(g):
        # 4 tensor_scalars per he-group; ONE tensor_tensor per PAIR of groups
        if g % 2 == 0 or g < 2:
            v = vp.tile([128, 2 * KG, R], f16, tag="v")
            v_tiles[g] = v
        else:
            v = v_tiles[g - 1]
            v_tiles[g] = v
        half = 0 if (g % 2 == 0 or g < 2) else 1
        if g < 2:
            half = 0
        vh = v[:, half * KG:(half + 1) * KG, :]
        for kk in range(KG):
            k = g * KG + kk
            nc.vector.tensor_scalar(
                vh[:, kk, :], c_rep, scl_d[:, k:k + 1], 1.0,
                mybir.AluOpType.mult, mybir.AluOpType.max,
            )
        if g < 2:
            # per-chunk TTs at the pipeline head: don't wait for the full quad
            for kk in range(KG):
                nc.vector.tensor_tensor(
                    vh[:, kk, :], vh[:, kk, :], m_tiles[g][:, kk, :],
                    mybir.AluOpType.mult,
                )
        elif g % 2 == 1:
            nc.vector.tensor_tensor(v, v, m_tiles[g], mybir.AluOpType.mult)

    def attention_pe(g):
        v = v_tiles[g]
        half = 0 if (g % 2 == 0 or g < 2) else 1
        first, last = ACC_ORDER[0], ACC_ORDER[-1]
        for kk in range(KG):
            k = g * KG + kk
            # PSUM bank limit: one matmul's output stays within 2KB/partition
            for hv in range(2):
                nc.tensor.matmul(
                    po[:, hv * 512:(hv + 1) * 512],
                    haug[:, k, 0:D + 2],
                    v[:, half * KG + kk, hv * 512:(hv + 1) * 512],
                    start=(g == first and kk == 0),
                    stop=(g == last and kk == KG - 1),
                )

    # ---- per-group pipeline
    for g in range(NG):
        xt = x_tiles[g]
        he = ps_he.tile([128, KG, HE_W], f32, tag="he")
        for kk in range(KG):
            for fc in range(FC):
                nc.tensor.matmul(
                    he[:, kk, :], xt[:, kk, fc, :], rhs_sb[:, fc, :],
                    start=(fc == 0), stop=(fc == FC - 1),
                )
        ks = slice(g * KG, (g + 1) * KG)
        # batched scalar-engine ACTs over the 4 chunks (strided he views)
        nc.scalar.activation(scl_d[:, ks], he[:, :, D + 1], Exp, scale=1.0 - ALPHA)
        nc.scalar.activation(scl_b2[:, ks], he[:, :, D + 1], Exp, scale=ALPHA)
        nc.scalar.activation(haug[:, ks, H_ONE], he[:, :, D + 1], Exp, scale=ALPHA)
        if g < 2:
            # per-chunk C so the c_rep diag chain starts before the batch ends
            for kk in range(KG):
                nc.scalar.activation(
                    scl_c[:, g * KG + kk:g * KG + kk + 1], he[:, kk, D:D + 1],
                    Exp, scale=1.0 - ALPHA,
                )
        for kk in range(KG):
            k = g * KG + kk
            # haug h columns = B2*h (per-partition scale AP)
            nc.scalar.activation(
                haug[:, k, 0:D], he[:, kk, 0:D], Ident, scale=scl_b2[:, k:k + 1]
            )

        if g == 1:
            # own chunks 0..7 done -> c_rep[p, r] = C[r] (broadcast across
            # partitions) via diag(C) matmul with an all-ones lhsT
            with tc.tile_pool(name="crep_tmp", bufs=1) as tmp, \
                 tc.tile_pool(name="crep_ps", bufs=1, space="PSUM") as tmps:
                cps = tmps.tile([128, R], f32)
                for rb in range(8):
                    dg = tmp.tile([128, 128], f16, tag="dg", bufs=2)
                    nc.vector.tensor_scalar(
                        dg, ident, scl_c[:, rb:rb + 1], None, mybir.AluOpType.mult
                    )
                    nc.tensor.matmul(
                        cps[:, rb * 128:(rb + 1) * 128], ones128, dg,
                        start=True, stop=True,
                    )
                nc.vector.tensor_copy(c_rep, cps)
            attention_dve(0)
            attention_dve(1)
        elif g >= 2:
            attention_dve(g)
            if g % 2 == 1:
                # PE accum burst for groups finished two steps back (keeps the
                # tensor engine in long uninterrupted runs); GPSIMD groups wait
                for gd in (g - 3, g - 2):
                    attention_pe(gd)
    for gd in (NG - 2, NG - 1):
        attention_pe(gd)

    # ---- normalize: out = numer * (1/denom)
    with tc.tile_pool(name="fin_ps", bufs=1, space="PSUM") as fps:
        # 1/d = exp(-ln(d)) on the scalar engine (denominator is positive)
        ln_row = outp.tile([1, R], f32)
        nc.scalar.activation(ln_row, po[D:D + 1, :], mybir.ActivationFunctionType.Ln)
        recip_row = outp.tile([1, R], f32)
        nc.scalar.activation(recip_row, ln_row, Exp, scale=-1.0)
        rr = fps.tile([D, R], f32)
        for hv in range(2):
            nc.tensor.matmul(
                rr[:, hv * 512:(hv + 1) * 512], ones_row_f,
                recip_row[:, hv * 512:(hv + 1) * 512], start=True, stop=True,
            )
        rr_sb = outp.tile([D, R], f32)
        nc.vector.tensor_copy(rr_sb, rr)
        o_t = outp.tile([D, R], f32)
        nc.vector.tensor_tensor(o_t, po[0:D, :], rr_sb, mybir.AluOpType.mult)
        for q in range(4):
            eng = nc.gpsimd if q % 2 == 0 else nc.sync
            eng.dma_start(
                out=outT[:, q * 256:(q + 1) * 256], in_=o_t[:, q * 256:(q + 1) * 256]
            )


def build_nc():
    nc = bacc.Bacc("TRN2", num_devices=N_CORES)
    xT4 = nc.dram_tensor("xT4", [NG, 128, KG, FC, 128], f16, kind="ExternalInput")
    maskT_rot = nc.dram_tensor("maskT_rot", [N, R], f16, kind="ExternalInput")
    rhs_f = nc.dram_tensor("rhs_f", [F_IN, HE_W], f16, kind="ExternalInput")
    outT = nc.dram_tensor("outT", [D, R], f32, kind="ExternalOutput")
    with ExitStack() as ctx:
        tc = ctx.enter_context(tile.TileContext(nc))
        build_kernel(ctx, tc, xT4[:, :, :, :, :], maskT_rot[:, :], rhs_f[:, :], outT[:, :])
    nc.compile()
    return nc


LAST_RESULTS = None


def kernel(x, mask, trans, attn, _trace=False):
    x = np.asarray(x, dtype=np.float32)
    mask = np.asarray(mask)
    trans = np.asarray(trans, dtype=np.float32)
    attn = np.asarray(attn, dtype=np.float32)

    x16 = np.ascontiguousarray(x, dtype=np.float16)
    # fused weights: h plus e1/e2 from one matmul ([trans | trans@a1 | trans@a2])
    ta12 = trans @ np.concatenate([attn[:D], attn[D:]], axis=1)  # [F_IN, 2]
    rhs_f = np.ascontiguousarray(
        np.concatenate([trans, ta12], axis=1), dtype=np.float16
    )
    maskT = np.ascontiguousarray(mask.T, dtype=np.float16)  # [N(j), N(r)] 0/1

    nc = build_nc()
    in_maps = []
    xc = x16.reshape(JC, 128, F_IN)
    mc = maskT.reshape(JC, 128, N)
    for c in range(N_CORES):
        # chunk rotation: own 8 chunks first, then the rest in order
        order = list(range(c * 8, c * 8 + 8)) + [
            k for k in range(JC) if not (c * 8 <= k < c * 8 + 8)
        ]
        # xT4[g][p][kk][fc][n] = x[chunk(4g+kk) node n, fc*128+p]
        xr = xc[order]  # [JC, 128(n), F_IN]
        xT4 = np.ascontiguousarray(
            xr.reshape(NG, KG, 128, FC, 128).transpose(0, 4, 1, 3, 2)
        )
        in_maps.append({
            "xT4": xT4,
            "maskT_rot": np.ascontiguousarray(
                mc[order, :, c * R:(c + 1) * R].reshape(N, R)
            ),
            "rhs_f": rhs_f,
        })
    res = run_bass_kernel_spmd(nc, in_maps, list(range(N_CORES)), trace=_trace)
    global LAST_RESULTS
    LAST_RESULTS = res
    out = np.concatenate(
        [res.results[c]["outT"].T for c in range(N_CORES)], axis=0
    )
    return np.ascontiguousarray(out, dtype=np.float32)


if __name__ == "__main__":
    nc = build_nc()
    print("built OK")


# revision 34
# speedup vs baseline: 1.0314x; 1.0314x over previous
"""Trainium2 Bass kernel for nn_NodeAttentionPerMetaPath (GAT-style node attention).

Reference computation (N=8192, F_IN=256, d=64):
    h      = x @ trans                      # [N, d]
    e1     = h @ attn[:d];  e2 = h @ attn[d:]
    scores = leaky_relu(e1 + e2.T, 0.2)     # [N, N]
    masked = where(mask==0, -1e15, scores)
    out    = softmax(masked, axis=1) @ h    # [N, d]

Sharding: rows (r) across 8 cores, 1024 rows each. Every core computes the
full h locally from a streamed copy of x (no collectives at all).

Algebra (exp of leaky_relu as a max of two exponentials; the exp(a*e1) factor
cancels in the softmax ratio):
    P'[r,j] = m[r,j] * max(C[r]*D[j], 1),  C = exp((1-a)e1), D = exp((1-a)e2)
    out[r]  = (sum_j P'[r,j]*B2[j]*h[j]) / (sum_j P'[r,j]*B2[j]),
    B2 = exp(a*e2); B2*h and B2 live as columns of one lhsT so a single
    accumulated PE matmul yields numerator AND denominator.

Device data flow is [j, r] so NO [N,N] transpose is ever needed on-device:
    - host uploads maskT (mask transposed, fp16 0/1): j lands on partitions
    - v[j,r] = max(C[r]*D[j], 1): one DVE tensor_scalar (4x 16-bit mode)
    - P'T    = v * maskT in place: one DVE/GPSIMD tensor_tensor (packed fp16)
    - out.T  = accumulated PE matmul over 64 j-chunks, lhsT = [B2*h | B2]

Host-side packing (lossless or quantization-only input repacking):
    - x -> xT4: fp16, pre-transposed into [g, p, kk, fc, n] 4-chunk groups so
      PE weight loads read it directly (no device transposes)
    - mask -> maskT fp16 (0/1 exact; halves mask DMA vs int32)
    - rhs_f = [trans | trans@attn] fp16: each x chunk yields h AND e1/e2 in
      one accumulated matmul pair
    - per-core chunk rotation: core c sees its OWN 8 node-chunks first (c_rep
      is needed early); maskT rows and haug slots use the same rotated j
      order, harmless since sum_j is order-invariant.
"""

from contextlib import ExitStack

import numpy as np

import concourse.bass as bass
import concourse.bacc as bacc
import concourse.mybir as mybir
import concourse.tile as tile
from concourse.bass_utils import run_bass_kernel_spmd
from concourse.masks import make_identity

f32 = mybir.dt.float32
f16 = mybir.dt.float16

Exp = mybir.ActivationFunctionType.Exp
Ident = mybir.ActivationFunctionType.Identity

N_CORES = 8
N = 8192
F_IN = 256
D = 64  # F_OUT
ALPHA = 0.2

R = N // N_CORES  # own rows per core
JC = N // 128  # j-chunks
FC = F_IN // 128  # f-chunks
KG = 4  # j-chunks per x/he group
NG = JC // KG

# haug columns: 0:64 = B2*h, 64 = B2 (denominator), 65 = zero pad
# (fp16 matmul lhsT needs an even element count)
H_ONE = D
H_W = D + 2
HE_W = D + 2  # he columns: 0:64 h, 64 e1, 65 e2


def build_kernel(ctx: ExitStack, tc: tile.TileContext, xT4, maskT_rot, rhs_f, outT):
    nc = tc.nc

    singles = ctx.enter_context(tc.tile_pool(name="singles", bufs=1))
    xp = ctx.enter_context(tc.tile_pool(name="xp", bufs=3))
    maskp = ctx.enter_context(tc.tile_pool(name="maskp", bufs=4))
    vp = ctx.enter_context(tc.tile_pool(name="vp", bufs=8))
    gvp = ctx.enter_context(tc.tile_pool(name="gvp", bufs=1))
    ps_he = ctx.enter_context(tc.tile_pool(name="ps_he", bufs=2, space="PSUM"))
    ps_o = ctx.enter_context(tc.tile_pool(name="ps_o", bufs=1, space="PSUM"))
    outp = ctx.enter_context(tc.tile_pool(name="outp", bufs=1))

    rhs_sb = singles.tile([128, FC, HE_W], f16)
    nc.gpsimd.dma_start(
        out=rhs_sb, in_=rhs_f.rearrange("(c p) d -> p c d", p=128)
    )

    # ---- interleaved input streams: xT group g (256KB) then its 4 maskT
    # tiles (256KB each) so a chunk's h is always ready before its mask.
    # own-row x groups 0/1 feed the c_rep critical path: issue their
    # per-chunk DMAs FIRST (the sync queue issues ~1 DMA per 700ns)
    x_tiles = []
    m_tiles = []
    for g in range(2):
        xt = xp.tile([128, KG, FC, 128], f16, tag="x")
        for kk in range(KG):
            nc.sync.dma_start(out=xt[:, kk, :, :], in_=xT4[g, :, kk])
        x_tiles.append(xt)
    for g in range(2):
        mt = maskp.tile([128, KG, R], f16, tag="m")
        for kk in range(KG):
            k = g * KG + kk
            nc.sync.dma_start(
                out=mt[:, kk, :], in_=maskT_rot[k * 128:(k + 1) * 128, :]
            )
        m_tiles.append(mt)
    for g in range(2, NG):
        xt = xp.tile([128, KG, FC, 128], f16, tag="x")
        nc.sync.dma_start(out=xt, in_=xT4[g])
        x_tiles.append(xt)
        mt = maskp.tile([128, KG, R], f16, tag="m")
        nc.sync.dma_start(
            out=mt,
            in_=maskT_rot[g * KG * 128:(g + 1) * KG * 128, :].rearrange(
                "(kk p) r -> p kk r", p=128
            ),
        )
        m_tiles.append(mt)

    # pin the natural_log_exp_and_others ACT table (id 6) at boot: it covers
    # every func used here (Exp/Identity/Copy/Ln) so no mid-run table swaps
    nc.scalar.add_instruction(
        mybir.InstLoadActFuncSet(
            name=nc.get_next_instruction_name(), ins=[], outs=[], act_func_set_id=6
        )
    )
    ident = singles.tile([128, 128], f16)
    make_identity(nc, ident)
    ones128 = singles.tile([128, 128], f16)
    nc.vector.memset(ones128, 1.0)
    ones_row_f = singles.tile([1, D], f32)
    nc.vector.memset(ones_row_f, 1.0)

    haug = singles.tile([128, JC, H_W], f16)
    nc.vector.memset(haug[:, :, H_ONE + 1], 0.0)
    # f32 per-partition scalars: D (for the tensor_scalar), B2 (ACT scale), C
    scl_d = singles.tile([128, JC], f32)
    scl_b2 = singles.tile([128, JC], f32)
    scl_c = singles.tile([128, 16], f32)
    c_rep = singles.tile([128, R], f16)

    po = ps_o.tile([D + 2, R], f32)

    v_tiles = {}
    GP_GROUPS = ()  # (gpsimd TT routing measured harmful; keep empty)
    ACC_ORDER = list(range(NG))

    def attention_dve(g):
        # one v quad per he-group: 4 tensor_scalars + ONE quad tensor_tensor
        v = vp.tile([128, KG, R], f16, tag="v")
        v_tiles[g] = v
        for kk in range(KG):
            k = g * KG + kk
            nc.vector.tensor_scalar(
                v[:, kk, :], c_rep, scl_d[:, k:k + 1], 1.0,
                mybir.AluOpType.mult, mybir.AluOpType.max,
            )
        if g < 2:
            # per-chunk TTs at the pipeline head: don't wait for the full quad
            for kk in range(KG):
                nc.vector.tensor_tensor(
                    v[:, kk, :], v[:, kk, :], m_tiles[g][:, kk, :],
                    mybir.AluOpType.mult,
                )
        else:
            nc.vector.tensor_tensor(v, v, m_tiles[g], mybir.AluOpType.mult)

    def attention_pe(g):
        v = v_tiles[g]
        first, last = ACC_ORDER[0], ACC_ORDER[-1]
        for kk in range(KG):
            k = g * KG + kk
            # PSUM bank limit: one matmul's output stays within 2KB/partition
            for hv in range(2):
                nc.tensor.matmul(
                    po[:, hv * 512:(hv + 1) * 512],
                    haug[:, k, 0:D + 2],
                    v[:, kk, hv * 512:(hv + 1) * 512],
                    start=(g == first and kk == 0),
                    stop=(g == last and kk == KG - 1),
                )

    # ---- per-group pipeline
    for g in range(NG):
        xt = x_tiles[g]
        he = ps_he.tile([128, KG, HE_W], f32, tag="he")
        for kk in range(KG):
            for fc in range(FC):
                nc.tensor.matmul(
                    he[:, kk, :], xt[:, kk, fc, :], rhs_sb[:, fc, :],
                    start=(fc == 0), stop=(fc == FC - 1),
                )
        ks = slice(g * KG, (g + 1) * KG)
        # batched scalar-engine ACTs over the 4 chunks (strided he views)
        nc.scalar.activation(scl_d[:, ks], he[:, :, D + 1], Exp, scale=1.0 - ALPHA)
        nc.scalar.activation(scl_b2[:, ks], he[:, :, D + 1], Exp, scale=ALPHA)
        nc.scalar.activation(haug[:, ks, H_ONE], he[:, :, D + 1], Exp, scale=ALPHA)
        if g < 2:
            # per-chunk C so the c_rep diag chain starts before the batch ends
            for kk in range(KG):
                nc.scalar.activation(
                    scl_c[:, g * KG + kk:g * KG + kk + 1], he[:, kk, D:D + 1],
                    Exp, scale=1.0 - ALPHA,
                )
        for kk in range(KG):
            k = g * KG + kk
            # haug h columns = B2*h (per-partition scale AP)
            nc.scalar.activation(
                haug[:, k, 0:D], he[:, kk, 0:D], Ident, scale=scl_b2[:, k:k + 1]
            )

        if g == 1:
            # own chunks 0..7 done -> c_rep[p, r] = C[r] (broadcast across
            # partitions) via diag(C) matmul with an all-ones lhsT
            with tc.tile_pool(name="crep_tmp", bufs=1) as tmp, \
                 tc.tile_pool(name="crep_ps", bufs=1, space="PSUM") as tmps:
                cps = tmps.tile([128, R], f32)
                for rb in range(8):
                    dg = tmp.tile([128, 128], f16, tag="dg", bufs=2)
                    nc.vector.tensor_scalar(
                        dg, ident, scl_c[:, rb:rb + 1], None, mybir.AluOpType.mult
                    )
                    nc.tensor.matmul(
                        cps[:, rb * 128:(rb + 1) * 128], ones128, dg,
                        start=True, stop=True,
                    )
                nc.vector.tensor_copy(c_rep, cps)
            attention_dve(0)
            attention_dve(1)
        elif g >= 2:
            attention_dve(g)
            if g % 2 == 1:
                # PE accum burst for groups finished two steps back (keeps the
                # tensor engine in long uninterrupted runs); GPSIMD groups wait
                for gd in (g - 3, g - 2):
                    attention_pe(gd)
    for gd in (NG - 2, NG - 1):
        attention_pe(gd)

    # ---- normalize: out = numer * (1/denom)
    with tc.tile_pool(name="fin_ps", bufs=1, space="PSUM") as fps:
        # 1/d = exp(-ln(d)) on the scalar engine (denominator is positive)
        ln_row = outp.tile([1, R], f32)
        nc.scalar.activation(ln_row, po[D:D + 1, :], mybir.ActivationFunctionType.Ln)
        recip_row = outp.tile([1, R], f32)
        nc.scalar.activation(recip_row, ln_row, Exp, scale=-1.0)
        rr = fps.tile([D, R], f32)
        for hv in range(2):
            nc.tensor.matmul(
                rr[:, hv * 512:(hv + 1) * 512], ones_row_f,
                recip_row[:, hv * 512:(hv + 1) * 512], start=True, stop=True,
            )
        rr_sb = outp.tile([D, R], f32)
        nc.vector.tensor_copy(rr_sb, rr)
        o_t = outp.tile([D, R], f32)
        nc.vector.tensor_tensor(o_t, po[0:D, :], rr_sb, mybir.AluOpType.mult)
        for q in range(4):
            eng = nc.gpsimd if q % 2 == 0 else nc.sync
            eng.dma_start(
                out=outT[:, q * 256:(q + 1) * 256], in_=o_t[:, q * 256:(q + 1) * 256]
            )


def build_nc():
    nc = bacc.Bacc("TRN2", num_devices=N_CORES)
    xT4 = nc.dram_tensor("xT4", [NG, 128, KG, FC, 128], f16, kind="ExternalInput")
    maskT_rot = nc.dram_tensor("maskT_rot", [N, R], f16, kind="ExternalInput")
    rhs_f = nc.dram_tensor("rhs_f", [F_IN, HE_W], f16, kind="ExternalInput")
    outT = nc.dram_tensor("outT", [D, R], f32, kind="ExternalOutput")
    with ExitStack() as ctx:
        tc = ctx.enter_context(tile.TileContext(nc))
        build_kernel(ctx, tc, xT4[:, :, :, :, :], maskT_rot[:, :], rhs_f[:, :], outT[:, :])
    nc.compile()
    return nc


LAST_RESULTS = None


def kernel(x, mask, trans, attn, _trace=False):
    x = np.asarray(x, dtype=np.float32)
    mask = np.asarray(mask)
    trans = np.asarray(trans, dtype=np.float32)
    attn = np.asarray(attn, dtype=np.float32)

    x16 = np.ascontiguousarray(x, dtype=np.float16)
    # fused weights: h plus e1/e2 from one matmul ([trans | trans@a1 | trans@a2])
    ta12 = trans @ np.concatenate([attn[:D], attn[D:]], axis=1)  # [F_IN, 2]
    rhs_f = np.ascontiguousarray(
        np.concatenate([trans, ta12], axis=1), dtype=np.float16
    )
    maskT = np.ascontiguousarray(mask.T, dtype=np.float16)  # [N(j), N(r)] 0/1

    nc = build_nc()
    in_maps = []
    xc = x16.reshape(JC, 128, F_IN)
    mc = maskT.reshape(JC, 128, N)
    for c in range(N_CORES):
        # chunk rotation: own 8 chunks first, then the rest in order
        order = list(range(c * 8, c * 8 + 8)) + [
            k for k in range(JC) if not (c * 8 <= k < c * 8 + 8)
        ]
        # xT4[g][p][kk][fc][n] = x[chunk(4g+kk) node n, fc*128+p]
        xr = xc[order]  # [JC, 128(n), F_IN]
        xT4 = np.ascontiguousarray(
            xr.reshape(NG, KG, 128, FC, 128).transpose(0, 4, 1, 3, 2)
        )
        in_maps.append({
            "xT4": xT4,
            "maskT_rot": np.ascontiguousarray(
                mc[order, :, c * R:(c + 1) * R].reshape(N, R)
            ),
            "rhs_f": rhs_f,
        })
    res = run_bass_kernel_spmd(nc, in_maps, list(range(N_CORES)), trace=_trace)
    global LAST_RESULTS
    LAST_RESULTS = res
    out = np.concatenate(
        [res.results[c]["outT"].T for c in range(N_CORES)], axis=0
    )
    return np.ascontiguousarray(out, dtype=np.float32)


if __name__ == "__main__":
    nc = build_nc()
    print("built OK")


# revision 35
# speedup vs baseline: 1.0394x; 1.0078x over previous
"""Trainium2 Bass kernel for nn_NodeAttentionPerMetaPath (GAT-style node attention).

Reference computation (N=8192, F_IN=256, d=64):
    h      = x @ trans                      # [N, d]
    e1     = h @ attn[:d];  e2 = h @ attn[d:]
    scores = leaky_relu(e1 + e2.T, 0.2)     # [N, N]
    masked = where(mask==0, -1e15, scores)
    out    = softmax(masked, axis=1) @ h    # [N, d]

Sharding: rows (r) across 8 cores, 1024 rows each. Every core computes the
full h locally from a streamed copy of x (no collectives at all).

Algebra (exp of leaky_relu as a max of two exponentials; the exp(a*e1) factor
cancels in the softmax ratio):
    P'[r,j] = m[r,j] * max(C[r]*D[j], 1),  C = exp((1-a)e1), D = exp((1-a)e2)
    out[r]  = (sum_j P'[r,j]*B2[j]*h[j]) / (sum_j P'[r,j]*B2[j]),
    B2 = exp(a*e2); B2*h and B2 live as columns of one lhsT so a single
    accumulated PE matmul yields numerator AND denominator.

Device data flow is [j, r] so NO [N,N] transpose is ever needed on-device:
    - host uploads maskT (mask transposed, fp16 0/1): j lands on partitions
    - v[j,r] = max(C[r]*D[j], 1): one DVE tensor_scalar (4x 16-bit mode)
    - P'T    = v * maskT in place: one DVE/GPSIMD tensor_tensor (packed fp16)
    - out.T  = accumulated PE matmul over 64 j-chunks, lhsT = [B2*h | B2]

Host-side packing (lossless or quantization-only input repacking):
    - x -> xT4: fp16, pre-transposed into [g, p, kk, fc, n] 4-chunk groups so
      PE weight loads read it directly (no device transposes)
    - mask -> maskT fp16 (0/1 exact; halves mask DMA vs int32)
    - rhs_f = [trans | trans@attn] fp16: each x chunk yields h AND e1/e2 in
      one accumulated matmul pair
    - per-core chunk rotation: core c sees its OWN 8 node-chunks first (c_rep
      is needed early); maskT rows and haug slots use the same rotated j
      order, harmless since sum_j is order-invariant.
"""

from contextlib import ExitStack

import numpy as np

import concourse.bass as bass
import concourse.bacc as bacc
import concourse.mybir as mybir
import concourse.tile as tile
from concourse.bass_utils import run_bass_kernel_spmd
from concourse.masks import make_identity

f32 = mybir.dt.float32
f16 = mybir.dt.float16

Exp = mybir.ActivationFunctionType.Exp
Ident = mybir.ActivationFunctionType.Identity

N_CORES = 8
N = 8192
F_IN = 256
D = 64  # F_OUT
ALPHA = 0.2

R = N // N_CORES  # own rows per core
JC = N // 128  # j-chunks
FC = F_IN // 128  # f-chunks
KG = 4  # j-chunks per x/he group
NG = JC // KG

# haug columns: 0:64 = B2*h, 64 = B2 (denominator), 65 = zero pad
# (fp16 matmul lhsT needs an even element count)
H_ONE = D
H_W = D + 2
HE_W = D + 2  # he columns: 0:64 h, 64 e1, 65 e2


def build_kernel(ctx: ExitStack, tc: tile.TileContext, xT4, maskT_rot, rhs_f, outT):
    nc = tc.nc

    singles = ctx.enter_context(tc.tile_pool(name="singles", bufs=1))
    xp = ctx.enter_context(tc.tile_pool(name="xp", bufs=5))
    maskp = ctx.enter_context(tc.tile_pool(name="maskp", bufs=6))
    vp = ctx.enter_context(tc.tile_pool(name="vp", bufs=8))
    gvp = ctx.enter_context(tc.tile_pool(name="gvp", bufs=1))
    ps_he = ctx.enter_context(tc.tile_pool(name="ps_he", bufs=2, space="PSUM"))
    ps_o = ctx.enter_context(tc.tile_pool(name="ps_o", bufs=1, space="PSUM"))
    outp = ctx.enter_context(tc.tile_pool(name="outp", bufs=1))

    rhs_sb = singles.tile([128, FC, HE_W], f16)
    nc.gpsimd.dma_start(
        out=rhs_sb, in_=rhs_f.rearrange("(c p) d -> p c d", p=128)
    )

    # ---- interleaved input streams: xT group g (256KB) then its 4 maskT
    # tiles (256KB each) so a chunk's h is always ready before its mask.
    # own-row x groups 0/1 feed the c_rep critical path: issue their
    # per-chunk DMAs FIRST (the sync queue issues ~1 DMA per 700ns)
    x_tiles = []
    m_tiles = []
    for g in range(2):
        xt = xp.tile([128, KG, FC, 128], f16, tag="x")
        for kk in range(KG):
            nc.sync.dma_start(out=xt[:, kk, :, :], in_=xT4[g, :, kk])
        x_tiles.append(xt)
    for g in range(2):
        mt = maskp.tile([128, KG, R], f16, tag="m")
        for kk in range(KG):
            k = g * KG + kk
            nc.sync.dma_start(
                out=mt[:, kk, :], in_=maskT_rot[k * 128:(k + 1) * 128, :]
            )
        m_tiles.append(mt)
    for g in range(2, NG):
        xt = xp.tile([128, KG, FC, 128], f16, tag="x")
        nc.sync.dma_start(out=xt, in_=xT4[g])
        x_tiles.append(xt)
        mt = maskp.tile([128, KG, R], f16, tag="m")
        nc.sync.dma_start(
            out=mt,
            in_=maskT_rot[g * KG * 128:(g + 1) * KG * 128, :].rearrange(
                "(kk p) r -> p kk r", p=128
            ),
        )
        m_tiles.append(mt)

    # pin the natural_log_exp_and_others ACT table (id 6) at boot: it covers
    # every func used here (Exp/Identity/Copy/Ln) so no mid-run table swaps
    nc.scalar.add_instruction(
        mybir.InstLoadActFuncSet(
            name=nc.get_next_instruction_name(), ins=[], outs=[], act_func_set_id=6
        )
    )
    ident = singles.tile([128, 128], f16)
    make_identity(nc, ident)
    ones128 = singles.tile([128, 128], f16)
    nc.vector.memset(ones128, 1.0)
    ones_row_f = singles.tile([1, D], f32)
    nc.vector.memset(ones_row_f, 1.0)

    haug = singles.tile([128, JC, H_W], f16)
    nc.vector.memset(haug[:, :, H_ONE + 1], 0.0)
    # f32 per-partition scalars: D (for the tensor_scalar), B2 (ACT scale), C
    scl_d = singles.tile([128, JC], f32)
    scl_b2 = singles.tile([128, JC], f32)
    scl_c = singles.tile([128, 16], f32)
    c_rep = singles.tile([128, R], f16)

    po = ps_o.tile([D + 2, R], f32)

    v_tiles = {}
    GP_GROUPS = ()  # (gpsimd TT routing measured harmful; keep empty)
    ACC_ORDER = list(range(NG))

    def attention_dve(g):
        # one v quad per he-group: 4 tensor_scalars + ONE quad tensor_tensor
        v = vp.tile([128, KG, R], f16, tag="v")
        v_tiles[g] = v
        for kk in range(KG):
            k = g * KG + kk
            nc.vector.tensor_scalar(
                v[:, kk, :], c_rep, scl_d[:, k:k + 1], 1.0,
                mybir.AluOpType.mult, mybir.AluOpType.max,
            )
        if g < 2:
            # per-chunk TTs at the pipeline head: don't wait for the full quad
            for kk in range(KG):
                nc.vector.tensor_tensor(
                    v[:, kk, :], v[:, kk, :], m_tiles[g][:, kk, :],
                    mybir.AluOpType.mult,
                )
        else:
            nc.vector.tensor_tensor(v, v, m_tiles[g], mybir.AluOpType.mult)

    def attention_pe(g):
        v = v_tiles[g]
        first, last = ACC_ORDER[0], ACC_ORDER[-1]
        for kk in range(KG):
            k = g * KG + kk
            # PSUM bank limit: one matmul's output stays within 2KB/partition
            for hv in range(2):
                nc.tensor.matmul(
                    po[:, hv * 512:(hv + 1) * 512],
                    haug[:, k, 0:D + 2],
                    v[:, kk, hv * 512:(hv + 1) * 512],
                    start=(g == first and kk == 0),
                    stop=(g == last and kk == KG - 1),
                )

    # ---- per-group pipeline
    for g in range(NG):
        xt = x_tiles[g]
        he = ps_he.tile([128, KG, HE_W], f32, tag="he")
        for kk in range(KG):
            for fc in range(FC):
                nc.tensor.matmul(
                    he[:, kk, :], xt[:, kk, fc, :], rhs_sb[:, fc, :],
                    start=(fc == 0), stop=(fc == FC - 1),
                )
        ks = slice(g * KG, (g + 1) * KG)
        # batched scalar-engine ACTs over the 4 chunks (strided he views)
        nc.scalar.activation(scl_d[:, ks], he[:, :, D + 1], Exp, scale=1.0 - ALPHA)
        nc.scalar.activation(scl_b2[:, ks], he[:, :, D + 1], Exp, scale=ALPHA)
        nc.scalar.activation(haug[:, ks, H_ONE], he[:, :, D + 1], Exp, scale=ALPHA)
        if g < 2:
            # per-chunk C so the c_rep diag chain starts before the batch ends
            for kk in range(KG):
                nc.scalar.activation(
                    scl_c[:, g * KG + kk:g * KG + kk + 1], he[:, kk, D:D + 1],
                    Exp, scale=1.0 - ALPHA,
                )
        for kk in range(KG):
            k = g * KG + kk
            # haug h columns = B2*h (per-partition scale AP)
            nc.scalar.activation(
                haug[:, k, 0:D], he[:, kk, 0:D], Ident, scale=scl_b2[:, k:k + 1]
            )

        if g == 1:
            # own chunks 0..7 done -> c_rep[p, r] = C[r] (broadcast across
            # partitions) via diag(C) matmul with an all-ones lhsT
            with tc.tile_pool(name="crep_tmp", bufs=1) as tmp, \
                 tc.tile_pool(name="crep_ps", bufs=1, space="PSUM") as tmps:
                cps = tmps.tile([128, R], f32)
                for rb in range(8):
                    dg = tmp.tile([128, 128], f16, tag="dg", bufs=2)
                    nc.vector.tensor_scalar(
                        dg, ident, scl_c[:, rb:rb + 1], None, mybir.AluOpType.mult
                    )
                    nc.tensor.matmul(
                        cps[:, rb * 128:(rb + 1) * 128], ones128, dg,
                        start=True, stop=True,
                    )
                nc.vector.tensor_copy(c_rep, cps)
            attention_dve(0)
            attention_dve(1)
        elif g >= 2:
            attention_dve(g)
            if g % 2 == 1:
                # PE accum burst for groups finished two steps back (keeps the
                # tensor engine in long uninterrupted runs); GPSIMD groups wait
                for gd in (g - 3, g - 2):
                    attention_pe(gd)
    for gd in (NG - 2, NG - 1):
        attention_pe(gd)

    # ---- normalize: out = numer * (1/denom)
    with tc.tile_pool(name="fin_ps", bufs=1, space="PSUM") as fps:
        # 1/d = exp(-ln(d)) on the scalar engine (denominator is positive)
        ln_row = outp.tile([1, R], f32)
        nc.scalar.activation(ln_row, po[D:D + 1, :], mybir.ActivationFunctionType.Ln)
        recip_row = outp.tile([1, R], f32)
        nc.scalar.activation(recip_row, ln_row, Exp, scale=-1.0)
        rr = fps.tile([D, R], f32)
        for hv in range(2):
            nc.tensor.matmul(
                rr[:, hv * 512:(hv + 1) * 512], ones_row_f,
                recip_row[:, hv * 512:(hv + 1) * 512], start=True, stop=True,
            )
        rr_sb = outp.tile([D, R], f32)
        nc.vector.tensor_copy(rr_sb, rr)
        o_t = outp.tile([D, R], f32)
        nc.vector.tensor_tensor(o_t, po[0:D, :], rr_sb, mybir.AluOpType.mult)
        for q in range(4):
            eng = nc.gpsimd if q % 2 == 0 else nc.sync
            eng.dma_start(
                out=outT[:, q * 256:(q + 1) * 256], in_=o_t[:, q * 256:(q + 1) * 256]
            )


def build_nc():
    nc = bacc.Bacc("TRN2", num_devices=N_CORES)
    xT4 = nc.dram_tensor("xT4", [NG, 128, KG, FC, 128], f16, kind="ExternalInput")
    maskT_rot = nc.dram_tensor("maskT_rot", [N, R], f16, kind="ExternalInput")
    rhs_f = nc.dram_tensor("rhs_f", [F_IN, HE_W], f16, kind="ExternalInput")
    outT = nc.dram_tensor("outT", [D, R], f32, kind="ExternalOutput")
    with ExitStack() as ctx:
        tc = ctx.enter_context(tile.TileContext(nc))
        build_kernel(ctx, tc, xT4[:, :, :, :, :], maskT_rot[:, :], rhs_f[:, :], outT[:, :])
    nc.compile()
    return nc


LAST_RESULTS = None


def kernel(x, mask, trans, attn, _trace=False):
    x = np.asarray(x, dtype=np.float32)
    mask = np.asarray(mask)
    trans = np.asarray(trans, dtype=np.float32)
    attn = np.asarray(attn, dtype=np.float32)

    x16 = np.ascontiguousarray(x, dtype=np.float16)
    # fused weights: h plus e1/e2 from one matmul ([trans | trans@a1 | trans@a2])
    ta12 = trans @ np.concatenate([attn[:D], attn[D:]], axis=1)  # [F_IN, 2]
    rhs_f = np.ascontiguousarray(
        np.concatenate([trans, ta12], axis=1), dtype=np.float16
    )
    maskT = np.ascontiguousarray(mask.T, dtype=np.float16)  # [N(j), N(r)] 0/1

    nc = build_nc()
    in_maps = []
    xc = x16.reshape(JC, 128, F_IN)
    mc = maskT.reshape(JC, 128, N)
    for c in range(N_CORES):
        # chunk rotation: own 8 chunks first, then the rest in order
        order = list(range(c * 8, c * 8 + 8)) + [
            k for k in range(JC) if not (c * 8 <= k < c * 8 + 8)
        ]
        # xT4[g][p][kk][fc][n] = x[chunk(4g+kk) node n, fc*128+p]
        xr = xc[order]  # [JC, 128(n), F_IN]
        xT4 = np.ascontiguousarray(
            xr.reshape(NG, KG, 128, FC, 128).transpose(0, 4, 1, 3, 2)
        )
        in_maps.append({
            "xT4": xT4,
            "maskT_rot": np.ascontiguousarray(
                mc[order, :, c * R:(c + 1) * R].reshape(N, R)
            ),
            "rhs_f": rhs_f,
        })
    res = run_bass_kernel_spmd(nc, in_maps, list(range(N_CORES)), trace=_trace)
    global LAST_RESULTS
    LAST_RESULTS = res
    out = np.concatenate(
        [res.results[c]["outT"].T for c in range(N_CORES)], axis=0
    )
    return np.ascontiguousarray(out, dtype=np.float32)


if __name__ == "__main__":
    nc = build_nc()
    print("built OK")
